# revision 4
# baseline (speedup 1.0000x reference)
"""MoE routing kernel for TRN2, SPMD over 8 NeuronCores.

Problem (per reference):
  x = mean(hidden_states, axis=1)                  # [B, H]
  scores = x @ gate_w + gate_b                     # [B, E]
  weights, sel = top_k(scores, 2)
  all_out = einsum('bh,eho->beo', x, expert_w) + expert_b
  out = sum(weights * all_out[b, sel], axis=1)     # [B, H]

Shapes: B=2048, S=256, H=1024, E=8, TOPK=2, fp32.

Strategy: data-parallel over batch (256 tokens/core), gate + 32 MB expert
weights replicated. Dense compute of all 8 experts per token, combined with
per-token mask weights m[b,e] = scores[b,e] if e in top2 else 0 — avoids any
gather/scatter. Memory-bound: 256 MB hidden_states stream per core dominates.

Per core:
  1. mean over S: DVE tensor_reduce on [128, CS, 1024] chunks (tokens on
     partitions), accumulate fp32.
  2. xT via PE transpose (experts contract over H, so x must be [H, tokens]).
  3. scores = xT.T @ gate_w (+ gate_b via K=1 ones-matmul); top-2 via
     reduce_max / select / reduce_max; m = scores * (scores >= max2).
  4. out init = m @ expert_b (PE, K=8); for each expert e: PSUM-accumulated
     fp32 matmuls over H, then out += m[:, e] * psum (DVE scalar_tensor_tensor).
"""

import numpy as np

B, S, H, E = 2048, 256, 1024, 8
N_CORES = 8
B_LOC = B // N_CORES          # 256 tokens per core
N_TT = B_LOC // 128           # 2 token-tiles of 128
CS = 8                        # s-values per hidden_states chunk
N_SC = S // CS                # 32 chunks per token-tile
KC = H // 128                 # 8 contraction chunks
NCH = H // 512                # 2 output column chunks

_compiled = None


def _build():
    import concourse.bacc as bacc
    import concourse.mybir as mybir
    import concourse.tile as tile
    from concourse.masks import make_identity

    fp32 = mybir.dt.float32
    nc = bacc.Bacc("TRN2", target_bir_lowering=False, debug=False,
                   num_devices=N_CORES)

    hs = nc.dram_tensor("hidden_states", [B_LOC, S, H], fp32,
                        kind="ExternalInput").ap()
    gate_w = nc.dram_tensor("gate_w", [H, E], fp32, kind="ExternalInput").ap()
    gate_b = nc.dram_tensor("gate_b", [E], fp32, kind="ExternalInput").ap()
    expert_w = nc.dram_tensor("expert_w", [E, H, H], fp32,
                              kind="ExternalInput").ap()
    expert_b = nc.dram_tensor("expert_b", [E, H], fp32,
                              kind="ExternalInput").ap()
    out = nc.dram_tensor("out", [B_LOC, H], fp32, kind="ExternalOutput").ap()

    with tile.TileContext(nc) as tc:
        with (
            tc.tile_pool(name="hs", bufs=2) as hs_pool,
            tc.tile_pool(name="w", bufs=16) as w_pool,
            tc.tile_pool(name="acc", bufs=1) as acc_pool,
            tc.tile_pool(name="small", bufs=1) as small_pool,
            tc.tile_pool(name="top2", bufs=1) as top2_pool,
            tc.tile_pool(name="psum", bufs=4, space="PSUM") as psum_pool,
            tc.tile_pool(name="psmall", bufs=2, space="PSUM") as psmall_pool,
        ):
            # --- constants / small inputs ---
            identity = small_pool.tile([128, 128], fp32, tag="ident")
            make_identity(nc, identity[:])
            ones_row = small_pool.tile([1, 128], fp32, tag="ones")
            nc.gpsimd.memset(ones_row[:], 1.0)

            gw_s = small_pool.tile([128, KC * E], fp32, tag="gw")  # [128, kc*E]
            for kc in range(KC):
                nc.gpsimd.dma_start(
                    out=gw_s[:, kc * E:(kc + 1) * E],
                    in_=gate_w[kc * 128:(kc + 1) * 128, :])
            gb_s = small_pool.tile([1, E], fp32, tag="gb")
            nc.gpsimd.dma_start(out=gb_s[:], in_=gate_b[None, :])
            eb_s = small_pool.tile([E, H], fp32, tag="eb")
            nc.gpsimd.dma_start(out=eb_s[:], in_=expert_b[:, :])

            # --- phase 1: mean over S (tokens on partitions) ---
            accs = []
            for tt in range(N_TT):
                acc = acc_pool.tile([128, H], fp32, tag=f"acc{tt}")
                for sc in range(N_SC):
                    chunk = hs_pool.tile([128, CS * H], fp32, tag="hs")
                    nc.sync.dma_start(
                        out=chunk[:].rearrange("p (s h) -> p s h", s=CS),
                        in_=hs[tt * 128:(tt + 1) * 128,
                               sc * CS:(sc + 1) * CS, :])
                    red_in = chunk[:].rearrange("p (s h) -> p h s", s=CS)
                    if sc == 0:
                        nc.vector.tensor_reduce(
                            acc[:], red_in, mybir.AxisListType.X,
                            mybir.AluOpType.add)
                    else:
                        partial = hs_pool.tile([128, H], fp32, tag="part")
                        nc.vector.tensor_reduce(
                            partial[:], red_in, mybir.AxisListType.X,
                            mybir.AluOpType.add)
                        nc.vector.tensor_add(acc[:], acc[:], partial[:])
                # x = acc / S
                nc.vector.tensor_scalar_mul(acc[:], acc[:], 1.0 / S)
                accs.append(acc)

            # --- phase 2: transpose x -> xT [H, B_loc] as KC tiles [128, B_loc]
            xT = []
            for kc in range(KC):
                xt = acc_pool.tile([128, B_LOC], fp32, tag=f"xt{kc}")
                xT.append(xt)
            for tt in range(N_TT):
                for kc in range(KC):
                    pt = psmall_pool.tile([128, 128], fp32, tag="pt")
                    nc.tensor.transpose(
                        pt[:], accs[tt][:, kc * 128:(kc + 1) * 128],
                        identity[:])
                    nc.vector.tensor_copy(
                        out=xT[kc][:, tt * 128:(tt + 1) * 128], in_=pt[:])

            # --- phase 3: gate scores + top-2 mask weights ---
            m_tiles = []   # [128, E] combine weights per token-tile
            mT_tiles = []  # [E, 128] transposed
            for tt in range(N_TT):
                ps_sc = psmall_pool.tile([128, E], fp32, tag="pt")
                for kc in range(KC):
                    nc.tensor.matmul(
                        ps_sc[:], xT[kc][:, tt * 128:(tt + 1) * 128],
                        gw_s[:, kc * E:(kc + 1) * E],
                        start=(kc == 0), stop=False)
                nc.tensor.matmul(ps_sc[:], ones_row[:], gb_s[:],
                                 start=False, stop=True)
                s_t = top2_pool.tile([128, E], fp32, tag=f"s{tt}")
                nc.vector.tensor_copy(out=s_t[:], in_=ps_sc[:])
                max1 = top2_pool.tile([128, 1], fp32, tag=f"mx1{tt}")
                nc.vector.tensor_reduce(
                    max1[:], s_t[:], mybir.AxisListType.X, mybir.AluOpType.max)
                ge1 = top2_pool.tile([128, E], fp32, tag=f"ge1{tt}")
                nc.vector.tensor_scalar(
                    ge1[:], s_t[:], max1[:], None, mybir.AluOpType.is_ge)
                masked = top2_pool.tile([128, E], fp32, tag=f"msk{tt}")
                nc.vector.scalar_tensor_tensor(
                    out=masked[:], in0=ge1[:], scalar=-1e30, in1=s_t[:],
                    op0=mybir.AluOpType.mult, op1=mybir.AluOpType.add)
                max2 = top2_pool.tile([128, 1], fp32, tag=f"mx2{tt}")
                nc.vector.tensor_reduce(
                    max2[:], masked[:], mybir.AxisListType.X,
                    mybir.AluOpType.max)
                ge2 = top2_pool.tile([128, E], fp32, tag=f"ge2{tt}")
                nc.vector.tensor_scalar(
                    ge2[:], s_t[:], max2[:], None, mybir.AluOpType.is_ge)
                m_t = top2_pool.tile([128, E], fp32, tag=f"m{tt}")
                nc.vector.tensor_mul(m_t[:], s_t[:], ge2[:])
                m_tiles.append(m_t)
                # transpose m -> mT [E, 128]
                pmT = psmall_pool.tile([E, 128], fp32, tag="pt")
                nc.tensor.transpose(pmT[:], m_t[:], identity[:])
                mT = top2_pool.tile([E, 128], fp32, tag=f"mT{tt}")
                nc.vector.tensor_copy(out=mT[:], in_=pmT[:])
                mT_tiles.append(mT)

            # --- phase 4: init out_acc with combined bias  m @ expert_b ---
            out_accs = []
            for tt in range(N_TT):
                oa = acc_pool.tile([128, H], fp32, tag=f"oa{tt}")
                for nch in range(NCH):
                    pb = psum_pool.tile([128, 512], fp32, tag="ps")
                    nc.tensor.matmul(
                        pb[:], mT_tiles[tt][:],
                        eb_s[:, nch * 512:(nch + 1) * 512],
                        start=True, stop=True)
                    nc.vector.tensor_copy(
                        out=oa[:, nch * 512:(nch + 1) * 512], in_=pb[:])
                out_accs.append(oa)

            # --- phase 5: experts ---
            for e in range(E):
                w_tiles = []
                for kc in range(KC):
                    wt = w_pool.tile([128, H], fp32, tag="w")
                    nc.gpsimd.dma_start(
                        out=wt[:],
                        in_=expert_w[e, kc * 128:(kc + 1) * 128, :])
                    w_tiles.append(wt)
                for tt in range(N_TT):
                    for nch in range(NCH):
                        ps = psum_pool.tile([128, 512], fp32, tag="ps")
                        for kc in range(KC):
                            nc.tensor.matmul(
                                ps[:], xT[kc][:, tt * 128:(tt + 1) * 128],
                                w_tiles[kc][:, nch * 512:(nch + 1) * 512],
                                start=(kc == 0), stop=(kc == KC - 1))
                        sl = out_accs[tt][:, nch * 512:(nch + 1) * 512]
                        nc.vector.scalar_tensor_tensor(
                            out=sl, in0=ps[:], scalar=m_tiles[tt][:, e:e + 1],
                            in1=sl, op0=mybir.AluOpType.mult,
                            op1=mybir.AluOpType.add)

            # --- phase 6: store ---
            for tt in range(N_TT):
                nc.sync.dma_start(out=out[tt * 128:(tt + 1) * 128, :],
                                  in_=out_accs[tt][:])

    nc.compile()
    return nc


def _get_compiled():
    global _compiled
    if _compiled is None:
        _compiled = _build()
    return _compiled


def kernel(**inputs):
    from concourse.bass_utils import run_bass_kernel_spmd

    hs = np.ascontiguousarray(np.asarray(inputs["hidden_states"],
                                         dtype=np.float32))
    gw = np.ascontiguousarray(np.asarray(inputs["gate_w"], dtype=np.float32))
    gb = np.ascontiguousarray(np.asarray(inputs["gate_b"], dtype=np.float32))
    ew = np.ascontiguousarray(np.asarray(inputs["expert_w"],
                                         dtype=np.float32))
    eb = np.ascontiguousarray(np.asarray(inputs["expert_b"],
                                         dtype=np.float32))

    nc = _get_compiled()
    in_maps = []
    for i in range(N_CORES):
        in_maps.append({
            "hidden_states": hs[i * B_LOC:(i + 1) * B_LOC],
            "gate_w": gw,
            "gate_b": gb,
            "expert_w": ew,
            "expert_b": eb,
        })
    res = run_bass_kernel_spmd(nc, in_maps, list(range(N_CORES)), trace=False)
    return np.concatenate([res.results[i]["out"] for i in range(N_CORES)],
                          axis=0)


# revision 8
# speedup vs baseline: 100.5330x; 100.5330x over previous
"""MoE routing kernel for TRN2, SPMD over 8 NeuronCores.

Problem (per reference):
  x = mean(hidden_states, axis=1)                  # [B, H]
  scores = x @ gate_w + gate_b                     # [B, E]
  weights, sel = top_k(scores, 2)
  all_out = einsum('bh,eho->beo', x, expert_w) + expert_b
  out = sum(weights * all_out[b, sel], axis=1)     # [B, H]

Shapes: B=2048, S=256, H=1024, E=8, TOPK=2, fp32.

Strategy: data-parallel over batch (256 tokens/core), gate + 32 MB expert
weights replicated. Dense compute of all 8 experts per token, combined with
per-token mask weights m[b,e] = scores[b,e] if e in top2 else 0 — avoids any
gather/scatter. Memory-bound: 256 MB hidden_states stream per core dominates.

Per core:
  1. mean over S: DVE tensor_reduce on [128, CS, 1024] chunks (tokens on
     partitions), accumulate fp32.
  2. xT via PE transpose (experts contract over H, so x must be [H, tokens]).
  3. scores = xT.T @ gate_w (+ gate_b via K=1 ones-matmul); top-2 via
     reduce_max / mask / reduce_max; m = scores * (scores >= max2).
  4. out init = m @ expert_b (PE, K=8); for each expert e: PSUM-accumulated
     fp32 matmuls over H, then out += m[:, e] * psum (DVE scalar_tensor_tensor).

`reps` repeats the whole compute inside one NEFF — used by the benchmark to
measure marginal (true device-side) kernel time past fixed dispatch overhead.
"""

import numpy as np

B, S, H, E = 2048, 256, 1024, 8
N_CORES = 8
B_LOC = B // N_CORES          # 256 tokens per core
N_TT = B_LOC // 128           # 2 token-tiles of 128
CS = 8                        # s-values per hidden_states chunk
N_SC = S // CS                # 32 chunks per token-tile
KC = H // 128                 # 8 contraction chunks
NCH = H // 512                # 2 output column chunks

_compiled = None


def _build(reps=1):
    import concourse.bacc as bacc
    import concourse.mybir as mybir
    import concourse.tile as tile
    from concourse.masks import make_identity

    fp32 = mybir.dt.float32
    nc = bacc.Bacc("TRN2", target_bir_lowering=False, debug=False,
                   num_devices=N_CORES)

    hs = nc.dram_tensor("hidden_states", [B_LOC, S, H], fp32,
                        kind="ExternalInput").ap()
    gate_w = nc.dram_tensor("gate_w", [H, E], fp32, kind="ExternalInput").ap()
    gate_b = nc.dram_tensor("gate_b", [E], fp32, kind="ExternalInput").ap()
    expert_w = nc.dram_tensor("expert_w", [E, H, H], fp32,
                              kind="ExternalInput").ap()
    expert_b = nc.dram_tensor("expert_b", [E, H], fp32,
                              kind="ExternalInput").ap()
    out = nc.dram_tensor("out", [B_LOC, H], fp32, kind="ExternalOutput").ap()

    with tile.TileContext(nc) as tc:
        with (
            tc.tile_pool(name="hs", bufs=2) as hs_pool,
            tc.tile_pool(name="w", bufs=16) as w_pool,
            tc.tile_pool(name="acc", bufs=1) as acc_pool,
            tc.tile_pool(name="small", bufs=1) as small_pool,
            tc.tile_pool(name="top2", bufs=1) as top2_pool,
            tc.tile_pool(name="psum", bufs=4, space="PSUM") as psum_pool,
            tc.tile_pool(name="psmall", bufs=2, space="PSUM") as psmall_pool,
        ):
            # --- constants / small inputs ---
            identity = small_pool.tile([128, 128], fp32, tag="ident")
            make_identity(nc, identity[:])
            ones_row = small_pool.tile([1, 128], fp32, tag="ones")
            nc.gpsimd.memset(ones_row[:], 1.0)

            gw_s = small_pool.tile([128, KC * E], fp32, tag="gw")  # [128, kc*E]
            for kc in range(KC):
                nc.gpsimd.dma_start(
                    out=gw_s[:, kc * E:(kc + 1) * E],
                    in_=gate_w[kc * 128:(kc + 1) * 128, :])
            gb_s = small_pool.tile([1, E], fp32, tag="gb")
            nc.gpsimd.dma_start(out=gb_s[:], in_=gate_b[None, :])
            eb_s = small_pool.tile([E, H], fp32, tag="eb")
            nc.gpsimd.dma_start(out=eb_s[:], in_=expert_b[:, :])

            import contextlib

            def body():
                # --- phase 1: mean over S (tokens on partitions) ---
                accs = []
                for tt in range(N_TT):
                    acc = acc_pool.tile([128, H], fp32, tag=f"acc{tt}")
                    for sc in range(N_SC):
                        chunk = hs_pool.tile([128, CS * H], fp32, tag="hs")
                        nc.sync.dma_start(
                            out=chunk[:].rearrange("p (s h) -> p s h", s=CS),
                            in_=hs[tt * 128:(tt + 1) * 128,
                                   sc * CS:(sc + 1) * CS, :])
                        red_in = chunk[:].rearrange("p (s h) -> p h s", s=CS)
                        if sc == 0:
                            nc.vector.tensor_reduce(
                                acc[:], red_in, mybir.AxisListType.X,
                                mybir.AluOpType.add)
                        else:
                            partial = hs_pool.tile([128, H], fp32, tag="part")
                            nc.vector.tensor_reduce(
                                partial[:], red_in, mybir.AxisListType.X,
                                mybir.AluOpType.add)
                            nc.vector.tensor_add(acc[:], acc[:], partial[:])
                    # x = acc / S
                    nc.vector.tensor_scalar_mul(acc[:], acc[:], 1.0 / S)
                    accs.append(acc)

                # --- phase 2: x -> xT [H, B_loc] as KC tiles [128, B_loc] ---
                xT = []
                for kc in range(KC):
                    xt = acc_pool.tile([128, B_LOC], fp32, tag=f"xt{kc}")
                    xT.append(xt)
                for tt in range(N_TT):
                    for kc in range(KC):
                        pt = psmall_pool.tile([128, 128], fp32, tag="pt")
                        nc.tensor.transpose(
                            pt[:], accs[tt][:, kc * 128:(kc + 1) * 128],
                            identity[:])
                        nc.vector.tensor_copy(
                            out=xT[kc][:, tt * 128:(tt + 1) * 128], in_=pt[:])

                # --- phase 3: gate scores + top-2 mask weights ---
                m_tiles = []   # [128, E] combine weights per token-tile
                mT_tiles = []  # [E, 128] transposed
                for tt in range(N_TT):
                    ps_sc = psmall_pool.tile([128, E], fp32, tag="pt")
                    for kc in range(KC):
                        nc.tensor.matmul(
                            ps_sc[:], xT[kc][:, tt * 128:(tt + 1) * 128],
                            gw_s[:, kc * E:(kc + 1) * E],
                            start=(kc == 0), stop=False)
                    nc.tensor.matmul(ps_sc[:], ones_row[:], gb_s[:],
                                     start=False, stop=True)
                    s_t = top2_pool.tile([128, E], fp32, tag=f"s{tt}")
                    nc.vector.tensor_copy(out=s_t[:], in_=ps_sc[:])
                    max1 = top2_pool.tile([128, 1], fp32, tag=f"mx1{tt}")
                    nc.vector.tensor_reduce(
                        max1[:], s_t[:], mybir.AxisListType.X,
                        mybir.AluOpType.max)
                    ge1 = top2_pool.tile([128, E], fp32, tag=f"ge1{tt}")
                    nc.vector.tensor_scalar(
                        ge1[:], s_t[:], max1[:], None, mybir.AluOpType.is_ge)
                    masked = top2_pool.tile([128, E], fp32, tag=f"msk{tt}")
                    nc.vector.scalar_tensor_tensor(
                        out=masked[:], in0=ge1[:], scalar=-1e30, in1=s_t[:],
                        op0=mybir.AluOpType.mult, op1=mybir.AluOpType.add)
                    max2 = top2_pool.tile([128, 1], fp32, tag=f"mx2{tt}")
                    nc.vector.tensor_reduce(
                        max2[:], masked[:], mybir.AxisListType.X,
                        mybir.AluOpType.max)
                    ge2 = top2_pool.tile([128, E], fp32, tag=f"ge2{tt}")
                    nc.vector.tensor_scalar(
                        ge2[:], s_t[:], max2[:], None, mybir.AluOpType.is_ge)
                    m_t = top2_pool.tile([128, E], fp32, tag=f"m{tt}")
                    nc.vector.tensor_mul(m_t[:], s_t[:], ge2[:])
                    m_tiles.append(m_t)
                    # transpose m -> mT [E, 128]
                    pmT = psmall_pool.tile([E, 128], fp32, tag="pt")
                    nc.tensor.transpose(pmT[:], m_t[:], identity[:])
                    mT = top2_pool.tile([E, 128], fp32, tag=f"mT{tt}")
                    nc.vector.tensor_copy(out=mT[:], in_=pmT[:])
                    mT_tiles.append(mT)

                # --- phase 4: init out_acc with combined bias m @ expert_b ---
                out_accs = []
                for tt in range(N_TT):
                    oa = acc_pool.tile([128, H], fp32, tag=f"oa{tt}")
                    for nch in range(NCH):
                        pb = psum_pool.tile([128, 512], fp32, tag="ps")
                        nc.tensor.matmul(
                            pb[:], mT_tiles[tt][:],
                            eb_s[:, nch * 512:(nch + 1) * 512],
                            start=True, stop=True)
                        nc.vector.tensor_copy(
                            out=oa[:, nch * 512:(nch + 1) * 512], in_=pb[:])
                    out_accs.append(oa)

                # --- phase 5: experts ---
                for e in range(E):
                    w_tiles = []
                    for kc in range(KC):
                        wt = w_pool.tile([128, H], fp32, tag="w")
                        nc.gpsimd.dma_start(
                            out=wt[:],
                            in_=expert_w[e, kc * 128:(kc + 1) * 128, :])
                        w_tiles.append(wt)
                    for tt in range(N_TT):
                        for nch in range(NCH):
                            ps = psum_pool.tile([128, 512], fp32, tag="ps")
                            for kc in range(KC):
                                nc.tensor.matmul(
                                    ps[:], xT[kc][:, tt * 128:(tt + 1) * 128],
                                    w_tiles[kc][:, nch * 512:(nch + 1) * 512],
                                    start=(kc == 0), stop=(kc == KC - 1))
                            sl = out_accs[tt][:, nch * 512:(nch + 1) * 512]
                            nc.vector.scalar_tensor_tensor(
                                out=sl, in0=ps[:],
                                scalar=m_tiles[tt][:, e:e + 1],
                                in1=sl, op0=mybir.AluOpType.mult,
                                op1=mybir.AluOpType.add)

                # --- phase 6: store ---
                for tt in range(N_TT):
                    nc.sync.dma_start(out=out[tt * 128:(tt + 1) * 128, :],
                                      in_=out_accs[tt][:])

            if reps == 1:
                body()
            else:
                with tc.For_i(0, reps, 1):
                    body()

    nc.compile()
    return nc


def _get_compiled():
    global _compiled
    if _compiled is None:
        _compiled = _build()
    return _compiled


def kernel(**inputs):
    from concourse.bass_utils import run_bass_kernel_spmd

    hs = np.ascontiguousarray(np.asarray(inputs["hidden_states"],
                                         dtype=np.float32))
    gw = np.ascontiguousarray(np.asarray(inputs["gate_w"], dtype=np.float32))
    gb = np.ascontiguousarray(np.asarray(inputs["gate_b"], dtype=np.float32))
    ew = np.ascontiguousarray(np.asarray(inputs["expert_w"],
                                         dtype=np.float32))
    eb = np.ascontiguousarray(np.asarray(inputs["expert_b"],
                                         dtype=np.float32))

    nc = _get_compiled()
    in_maps = []
    for i in range(N_CORES):
        in_maps.append({
            "hidden_states": hs[i * B_LOC:(i + 1) * B_LOC],
            "gate_w": gw,
            "gate_b": gb,
            "expert_w": ew,
            "expert_b": eb,
        })
    res = run_bass_kernel_spmd(nc, in_maps, list(range(N_CORES)), trace=False)
    return np.concatenate([res.results[i]["out"] for i in range(N_CORES)],
                          axis=0)
